# revision 4
# baseline (speedup 1.0000x reference)
"""Trainium2 Bass kernel for BoundaryFocalLoss.

Full-input contract: kernel(**inputs) takes the complete arrays
(inputs [128,200000] f32, targets [128,200000] i32, mask [128,200000] f32)
and returns the scalar loss, distributing work over 8 NeuronCores by
sharding the T dimension (each core: all 128 batch rows x 25000 columns;
targets carry a 5/3-column halo for the 7-wide boundary window).

Math (signed-logit form, equivalent to the reference):
    e    = exp(-x)                      # bf16 x (high half of f32 via strided DMA)
    Lm   = ln(1+e)       = softplus(-x)
    sigma= exp(-Lm)      = sigmoid(x)
    bce  = Lm + x*(0.975 - 0.95*t)      # = relu(x) - s*x + ln(1+exp(-|x|))
    pt   = exp(-bce)
    ada  = 1 - |sigma - 0.5|
    W    = 1 + 4*dilate7(transitions)   # via prefix-count scan + shifted compare
    aw   = 0.75 - 0.5*t
    F    = ada*(1-pt)^2 * W*aw * bce
    loss = sum(F * mask) / sum(mask)

Engine split per tile: 4 activation passes (one act-table set), lean DVE
pipeline with two custom fused DVE ops (transition prefix-count scan in
fp16; ada*(1-pt)^2), the W*aw product on the Pool engine, and the final
spatial reduction on the TensorEngine as a ones-weight matmul accumulated
in PSUM across tiles.

Inputs reach the device untouched: x is DMA'd as the high 2 bytes of each
f32 (bf16 truncation done by the DMA access pattern), targets as the low
2 bytes of each int32 (values 0/1 are int16-exact). The host only
reinterprets (ndarray.view) - no host-side arithmetic or conversion.
"""

import numpy as np
import ml_dtypes
from contextlib import ExitStack

P = 128          # partitions == batch rows
N_CORES = 8
HALO_L, HALO_R = 5, 3
HALO = HALO_L + HALO_R   # 8


def _register_custom_ops():
    """Register the two fused DVE ops in concourse's custom-DVE registry
    (runtime equivalent of appending to dve_ops.OPS per the authoring guide)."""
    import concourse.dve_ops as dve_ops
    from concourse.dve_spec import (Spec, Src0, Src1, C0, Zero, One, maxx, sq,
                                    ne, scan, AluOp, lower)
    from concourse.dve_uop import DveOpSpec

    existing = {op.name for op in dve_ops.OPS}

    def add(name, spec):
        if name in existing:
            return
        uops = lower(spec, ver="v3")
        sha = DveOpSpec(name=name, uops=uops, rd1_en=True).sha("v3")
        op = dve_ops.DveOp(name, spec, subdim=False, uops_sha={"v3": sha})
        dve_ops.OPS.append(op)
        dve_ops._SUB_OPCODE_FOR_NAME[name] = (
            dve_ops._CUSTOM_DVE_ROW_BASE + len(dve_ops.OPS) - 1)
        dve_ops.CUSTOM_DVE_SPECS[name] = spec

    add("BFL_SCAN_NE", Spec(
        body=scan(AluOp.ADD, ne(Src0, Src1)),
        reference=lambda in0, in1, s0, s1, imm2:
            np.cumsum((in0 != in1).astype(np.float32), axis=-1)
            .astype(np.float32)))

    _q = Src0 - C0
    _a = maxx(_q, Zero - _q)
    add("BFL_ADA_OMP2", Spec(
        body=(One - _a) * sq(One - Src1),
        reference=lambda in0, in1, s0, s1, imm2:
            ((1.0 - np.abs(in0 - s0)) * (1.0 - in1) ** 2).astype(np.float32)))


def _get_custom(name):
    import concourse.dve_ops as dve_ops
    return next(o for o in dve_ops.OPS if o.name == name)


def _build_program(T_shard, N, with_mask):
    """Build + compile the single-core Bass program (SPMD across cores)."""
    import concourse.bacc as bacc
    import concourse.tile as tile
    import concourse.mybir as mybir

    _register_custom_ops()
    scan_op = _get_custom("BFL_SCAN_NE")
    ada_op = _get_custom("BFL_ADA_OMP2")

    dt = mybir.dt
    Alu = mybir.AluOpType
    Act = mybir.ActivationFunctionType

    NT = T_shard // N
    assert NT * N == T_shard
    assert N + HALO - 1 <= 2047  # fp16-exact prefix counts in the scan

    # Single act-table set (Exp/Ln co-resident) to avoid table thrash.
    import concourse.hw_specs as hw_specs
    import bass_rust as _bass_rust

    _ONE_SET = "natural_log_exp_and_others"
    _USED = {Act.Exp, Act.Ln, Act.Copy, Act.Identity}

    class _OneActSetBacc(bacc.Bacc):
        def insert_act_table_loads(self):
            has_activation = any(
                isinstance(i, mybir.InstActivation)
                for b in self.main_func.blocks
                for i in b.instructions
            )
            if not has_activation:
                return
            tables = [
                (name, (funcs if name == _ONE_SET else funcs - _USED))
                for name, funcs in hw_specs.get_activation_tables(self.m.arch).items()
            ]
            _bass_rust.insert_act_table_loads(self, tables)

    nc = _OneActSetBacc("TRN2", target_bir_lowering=False, debug=False)

    # x as bf16 pairs (high half = bf16 truncation), t as int16 pairs (low half)
    x_d = nc.dram_tensor("x", [P, 2 * T_shard], dt.bfloat16,
                         kind="ExternalInput").ap()
    t_d = nc.dram_tensor("t", [P, 2 * (T_shard + HALO)], dt.int16,
                         kind="ExternalInput").ap()
    ones_d = nc.dram_tensor("ones", [P, 1], dt.bfloat16,
                            kind="ExternalInput").ap()
    if with_mask:
        m_d = nc.dram_tensor("m", [P, 2 * T_shard], dt.bfloat16,
                             kind="ExternalInput").ap()
    out_d = nc.dram_tensor("out", [P, 2], dt.float32, kind="ExternalOutput").ap()

    # PSUM chunking for the ones-matmul reduction (bank = 512 f32)
    chunks = []
    off = 0
    while off < N:
        w = min(500, N - off)
        chunks.append((off, w))
        off += w

    with tile.TileContext(nc) as tc, ExitStack() as ctx:
        io = ctx.enter_context(tc.tile_pool(name="io", bufs=3))
        val = ctx.enter_context(tc.tile_pool(name="val", bufs=2))
        singles = ctx.enter_context(tc.tile_pool(name="singles", bufs=1))
        psum = ctx.enter_context(tc.tile_pool(name="psum", bufs=1, space="PSUM"))

        ones_sb = singles.tile([P, 1], dt.bfloat16)
        nc.sync.dma_start(ones_sb[:], ones_d[:])
        out_sb = singles.tile([P, 2], dt.float32)
        nc.vector.memset(out_sb[:], 0.0)
        if with_mask:
            ms = singles.tile([P, NT], dt.float32)

        ps = [psum.tile([1, w], dt.float32, name=f"ps{ci}", tag=f"ps{ci}")
              for ci, (_, w) in enumerate(chunks)]

        for i in range(NT):
            c0 = i * N
            # ---- strided loads (dtype extraction by DMA) ---------------
            xb = io.tile([P, N], dt.bfloat16, tag="x")
            nc.sync.dma_start(xb[:], x_d[:, 2 * c0 + 1: 2 * (c0 + N): 2])
            tcb = io.tile([P, N + HALO], dt.int16, tag="t")
            nc.sync.dma_start(tcb[:], t_d[:, 2 * c0: 2 * (c0 + N + HALO): 2])
            if with_mask:
                mb = io.tile([P, N], dt.bfloat16, tag="m")
                nc.sync.dma_start(mb[:], m_d[:, 2 * c0 + 1: 2 * (c0 + N): 2])
            tc_c = tcb[:, HALO_L:HALO_L + N]

            # ---- ACT chain (scalar engine) -----------------------------
            e = val.tile([P, N], dt.bfloat16, tag="e")
            nc.scalar.activation(e[:], xb[:], Act.Exp, scale=-1.0)
            Lm = val.tile([P, N], dt.bfloat16, tag="Lm")
            nc.scalar.activation(Lm[:], e[:], Act.Ln, bias=1.0)
            sg = val.tile([P, N], dt.bfloat16, tag="sg")
            nc.scalar.activation(sg[:], Lm[:], Act.Exp, scale=-1.0)

            # ---- bce ----------------------------------------------------
            s_t = val.tile([P, N], dt.bfloat16, tag="s")
            nc.vector.tensor_scalar(s_t[:], tc_c, -0.95, 0.975, Alu.mult, Alu.add)
            xst = val.tile([P, N], dt.bfloat16, tag="xst")
            nc.vector.tensor_tensor(xst[:], xb[:], s_t[:], Alu.mult)
            bce = val.tile([P, N], dt.bfloat16, tag="bce")
            nc.vector.tensor_tensor(bce[:], xst[:], Lm[:], Alu.add)
            pt = val.tile([P, N], dt.bfloat16, tag="pt")
            nc.scalar.activation(pt[:], bce[:], Act.Exp, scale=-1.0)

            # ---- boundary dilation (prefix-count scan + shifted ne) ----
            TRS = val.tile([P, N + HALO - 1], dt.float16, tag="TRS")
            nc.vector._custom_dve(scan_op, out=TRS[:],
                                  in0=tcb[:, 1:N + HALO],
                                  in1=tcb[:, 0:N + HALO - 1])
            d3 = val.tile([P, N], dt.bfloat16, tag="d3")
            nc.vector.tensor_tensor(d3[:], TRS[:, 7:N + 7], TRS[:, 0:N],
                                    Alu.not_equal)
            W = val.tile([P, N], dt.bfloat16, tag="W")
            nc.vector.tensor_scalar(W[:], d3[:], 4.0, 1.0, Alu.mult, Alu.add)
            aw = val.tile([P, N], dt.bfloat16, tag="aw")
            nc.vector.tensor_scalar(aw[:], tc_c, -0.5, 0.75, Alu.mult, Alu.add)
            Waw = val.tile([P, N], dt.bfloat16, tag="Waw")
            nc.gpsimd.tensor_tensor(Waw[:], W[:], aw[:], Alu.mult)

            # ---- focal factors -----------------------------------------
            AOO = val.tile([P, N], dt.bfloat16, tag="AOO")
            nc.vector._custom_dve(ada_op, out=AOO[:], in0=sg[:], in1=pt[:],
                                  s0=0.5)
            lhs = val.tile([P, N], dt.bfloat16, tag="lhs")
            nc.vector.tensor_tensor(lhs[:], AOO[:], Waw[:], Alu.mult)
            F = val.tile([P, N], dt.bfloat16, tag="F")
            nc.vector.tensor_tensor(F[:], lhs[:], bce[:], Alu.mult)
            if with_mask:
                Fm = val.tile([P, N], dt.bfloat16, tag="Fm")
                nc.vector.tensor_tensor(Fm[:], F[:], mb[:], Alu.mult)
                F = Fm
                nc.vector.tensor_reduce(
                    ms[:, i:i + 1], mb[:], axis=mybir.AxisListType.X, op=Alu.add)

            # ---- TensorEngine reduction: ones^T @ F chunks -------------
            for c, (coff, w) in enumerate(chunks):
                nc.tensor.matmul(ps[c][0:1, 0:w], ones_sb[:, 0:1],
                                 F[:, coff:coff + w],
                                 start=(i == 0), stop=(i == NT - 1))

        # ---- tail: spill psum, reduce to scalar ------------------------
        tailw = sum(w for (_, w) in chunks)
        tail = singles.tile([1, tailw], dt.float32)
        off = 0
        for c, (coff, w) in enumerate(chunks):
            nc.vector.tensor_copy(tail[0:1, off:off + w], ps[c][0:1, 0:w])
            off += w
        nc.vector.tensor_reduce(out_sb[0:1, 0:1], tail[0:1, :],
                                axis=mybir.AxisListType.X, op=Alu.add)
        if with_mask:
            nc.vector.tensor_reduce(
                out_sb[:, 1:2], ms[:], axis=mybir.AxisListType.X, op=Alu.add)
        nc.sync.dma_start(out_d[:], out_sb[:])

    nc.compile()
    return nc


_PROGRAM_CACHE = {}


def _get_program(T_shard, N=1250, with_mask=False):
    key = (T_shard, N, with_mask)
    if key not in _PROGRAM_CACHE:
        _PROGRAM_CACHE[key] = _build_program(T_shard, N, with_mask)
    return _PROGRAM_CACHE[key]


def _make_in_maps(x, t, m=None):
    """Per-core input dicts. Only ndarray.view reinterpretation on host."""
    Bq, T = x.shape
    T_shard = T // N_CORES
    t_pad = np.pad(t, ((0, 0), (HALO_L, HALO_R)), mode="edge")
    ones = np.ones((P, 1), dtype=ml_dtypes.bfloat16)
    in_maps = []
    for c in range(N_CORES):
        lo = c * T_shard
        im = {
            "x": np.ascontiguousarray(x[:, lo:lo + T_shard]).view(ml_dtypes.bfloat16),
            "t": np.ascontiguousarray(t_pad[:, lo:lo + T_shard + HALO]).view(np.int16),
            "ones": ones,
        }
        if m is not None:
            im["m"] = np.ascontiguousarray(m[:, lo:lo + T_shard]).view(ml_dtypes.bfloat16)
        in_maps.append(im)
    return in_maps


def kernel(inputs, targets, mask):
    from concourse.bass_utils import run_bass_kernel_spmd

    x = np.ascontiguousarray(np.asarray(inputs, dtype=np.float32))
    t = np.ascontiguousarray(np.asarray(targets, dtype=np.int32))
    m = np.ascontiguousarray(np.asarray(mask, dtype=np.float32))
    Bq, T = x.shape
    assert Bq == P and T % N_CORES == 0
    T_shard = T // N_CORES
    ones_mask = bool(m.min() == 1.0 and m.max() == 1.0)

    nc = _get_program(T_shard, 1250, with_mask=not ones_mask)
    in_maps = _make_in_maps(x, t, None if ones_mask else m)

    res = run_bass_kernel_spmd(nc, in_maps, core_ids=list(range(N_CORES)))
    outs = [r["out"] for r in res.results]

    loss = float(sum(float(o[0, 0]) for o in outs))
    if ones_mask:
        msum = float(Bq) * float(T)
    else:
        msum = float(sum(o[:, 1].astype(np.float64).sum() for o in outs))
    if msum <= 0.0:
        return np.float32(0.0)
    return np.float32(loss / msum)


# revision 11
# speedup vs baseline: 26.4710x; 26.4710x over previous
"""Trainium2 Bass kernel for BoundaryFocalLoss.

Full-input contract: kernel(**inputs) takes the complete arrays
(inputs [128,200000] f32, targets [128,200000] i32, mask [128,200000] f32)
and returns the scalar loss, distributing work over 8 NeuronCores by
sharding the T dimension (each core: all 128 batch rows x 25000 columns;
targets carry a 5/3-column halo for the 7-wide boundary window).

Math (signed-logit form, equivalent to the reference):
    e    = exp(-x)                      # bf16 x (high half of f32 via strided DMA)
    Lm   = ln(1+e)       = softplus(-x)
    sigma= exp(-Lm)      = sigmoid(x)
    bce  = Lm + x*(0.975 - 0.95*t)      # = relu(x) - s*x + ln(1+exp(-|x|))
    pt   = exp(-bce)
    ada  = 1 - |sigma - 0.5|
    W    = 1 + 4*dilate7(transitions)   # via prefix-count scan + shifted compare
    aw   = 0.75 - 0.5*t
    F    = ada*(1-pt)^2 * W*aw * bce
    loss = sum(F * mask) / sum(mask)

Engine split per tile: 4 activation passes (one act-table set), lean DVE
pipeline with two custom fused DVE ops (transition prefix-count scan in
fp16; ada*(1-pt)^2), the W*aw product on the Pool engine, and the final
spatial reduction on the TensorEngine as a ones-weight matmul accumulated
in PSUM across tiles.

Inputs reach the device untouched: x is DMA'd as the high 2 bytes of each
f32 (bf16 truncation done by the DMA access pattern), targets as the low
2 bytes of each int32 (values 0/1 are int16-exact). The host only
reinterprets (ndarray.view) - no host-side arithmetic or conversion.
"""

import numpy as np
import ml_dtypes
from contextlib import ExitStack

P = 128          # partitions == batch rows
N_CORES = 8
HALO_L, HALO_R = 5, 3
HALO = HALO_L + HALO_R   # 8


def _register_custom_ops():
    """Register the two fused DVE ops in concourse's custom-DVE registry
    (runtime equivalent of appending to dve_ops.OPS per the authoring guide)."""
    import concourse.dve_ops as dve_ops
    from concourse.dve_spec import (Spec, Src0, Src1, C0, C1, Zero, One, maxx,
                                    sq, ne, scan, AluOp, lower)
    from concourse.dve_uop import DveOpSpec

    existing = {op.name for op in dve_ops.OPS}

    def add(name, spec):
        if name in existing:
            return
        uops = lower(spec, ver="v3")
        sha = DveOpSpec(name=name, uops=uops, rd1_en=True).sha("v3")
        op = dve_ops.DveOp(name, spec, subdim=False, uops_sha={"v3": sha})
        dve_ops.OPS.append(op)
        dve_ops._SUB_OPCODE_FOR_NAME[name] = (
            dve_ops._CUSTOM_DVE_ROW_BASE + len(dve_ops.OPS) - 1)
        dve_ops.CUSTOM_DVE_SPECS[name] = spec

    add("BFL_SCAN_NE", Spec(
        body=scan(AluOp.ADD, ne(Src0, Src1)),
        reference=lambda in0, in1, s0, s1, imm2:
            np.cumsum((in0 != in1).astype(np.float32), axis=-1)
            .astype(np.float32)))

    _q = Src0 - C0
    _a = maxx(_q, Zero - _q)
    add("BFL_ADA_OMP2", Spec(
        body=(One - _a) * sq(One - Src1),
        reference=lambda in0, in1, s0, s1, imm2:
            ((1.0 - np.abs(in0 - s0)) * (1.0 - in1) ** 2).astype(np.float32)))

    # xst = x * (c1 - c0*t): reads the f32 x and i32 t tiles through strided
    # bitcast views (bf16 high half / int16 low half) - customs run at
    # 1 elem/cycle so strided inputs cost nothing extra.
    add("BFL_XST", Spec(
        body=Src0 * (C1 - Src1 * C0),
        reference=lambda in0, in1, s0, s1, imm2:
            (in0.astype(np.float32) * (s1 - in1 * s0)).astype(np.float32)))


def _get_custom(name):
    import concourse.dve_ops as dve_ops
    return next(o for o in dve_ops.OPS if o.name == name)


def _build_program(T_shard, N, with_mask):
    """Build + compile the single-core Bass program (SPMD across cores)."""
    import concourse.bacc as bacc
    import concourse.tile as tile
    import concourse.mybir as mybir

    _register_custom_ops()
    scan_op = _get_custom("BFL_SCAN_NE")
    ada_op = _get_custom("BFL_ADA_OMP2")
    xst_op = _get_custom("BFL_XST")

    dt = mybir.dt
    Alu = mybir.AluOpType
    Act = mybir.ActivationFunctionType

    NT = T_shard // N
    assert NT * N == T_shard
    assert N + HALO - 1 <= 2047  # fp16-exact prefix counts in the scan

    # Single act-table set (Exp/Ln co-resident) to avoid table thrash.
    import concourse.hw_specs as hw_specs
    import bass_rust as _bass_rust

    _ONE_SET = "natural_log_exp_and_others"
    _USED = {Act.Exp, Act.Ln, Act.Copy, Act.Identity}

    class _OneActSetBacc(bacc.Bacc):
        def insert_act_table_loads(self):
            has_activation = any(
                isinstance(i, mybir.InstActivation)
                for b in self.main_func.blocks
                for i in b.instructions
            )
            if not has_activation:
                return
            tables = [
                (name, (funcs if name == _ONE_SET else funcs - _USED))
                for name, funcs in hw_specs.get_activation_tables(self.m.arch).items()
            ]
            _bass_rust.insert_act_table_loads(self, tables)

    nc = _OneActSetBacc("TRN2", target_bir_lowering=False, debug=False)

    x_d = nc.dram_tensor("x", [P, T_shard], dt.float32,
                         kind="ExternalInput").ap()
    t_d = nc.dram_tensor("t", [P, T_shard + HALO], dt.int32,
                         kind="ExternalInput").ap()
    ones_d = nc.dram_tensor("ones", [P, 1], dt.bfloat16,
                            kind="ExternalInput").ap()
    if with_mask:
        m_d = nc.dram_tensor("m", [P, T_shard], dt.float32,
                             kind="ExternalInput").ap()
    out_d = nc.dram_tensor("out", [P, 2], dt.float32, kind="ExternalOutput").ap()

    # PSUM chunking for the ones-matmul reduction (bank = 512 f32)
    chunks = []
    off = 0
    while off < N:
        w = min(500, N - off)
        chunks.append((off, w))
        off += w

    with tile.TileContext(nc) as tc, ExitStack() as ctx:
        io = ctx.enter_context(tc.tile_pool(name="io", bufs=3))
        val = ctx.enter_context(tc.tile_pool(name="val", bufs=2))
        singles = ctx.enter_context(tc.tile_pool(name="singles", bufs=1))
        psum = ctx.enter_context(tc.tile_pool(name="psum", bufs=1, space="PSUM"))

        ones_sb = singles.tile([P, 1], dt.bfloat16)
        nc.sync.dma_start(ones_sb[:], ones_d[:])
        out_sb = singles.tile([P, 2], dt.float32)
        nc.vector.memset(out_sb[:], 0.0)
        if with_mask:
            ms = singles.tile([P, NT], dt.float32)

        ps = [psum.tile([1, w], dt.float32, name=f"ps{ci}", tag=f"ps{ci}")
              for ci, (_, w) in enumerate(chunks)]

        for i in range(NT):
            c0 = i * N
            # ---- contiguous loads; dtype views happen at the consumers -
            xt = io.tile([P, N], dt.float32, tag="x")
            nc.sync.dma_start(xt[:], x_d[:, c0:c0 + N])
            tt = io.tile([P, N + HALO], dt.int32, tag="t")
            nc.sync.dma_start(tt[:], t_d[:, c0:c0 + N + HALO])
            if with_mask:
                mt = io.tile([P, N], dt.float32, tag="m")
                nc.sync.dma_start(mt[:], m_d[:, c0:c0 + N])
            # strided views: bf16 high half of f32, int16 low half of i32
            xb = xt[:].bitcast(dt.bfloat16)[:, 1::2]
            tcv = tt[:].bitcast(dt.int16)[:, 0::2]
            tc_c = tcv[:, HALO_L:HALO_L + N]

            # ---- ACT chain (scalar engine reads strided at no cost) ----
            e = val.tile([P, N], dt.bfloat16, tag="e")
            nc.scalar.activation(e[:], xb, Act.Exp, scale=-1.0)
            Lm = val.tile([P, N], dt.bfloat16, tag="Lm")
            nc.scalar.activation(Lm[:], e[:], Act.Ln, bias=1.0)
            sg = val.tile([P, N], dt.bfloat16, tag="sg")
            nc.scalar.activation(sg[:], Lm[:], Act.Exp, scale=-1.0)

            # ---- bce = Lm + x*(0.975-0.95t) ----------------------------
            xst = val.tile([P, N], dt.bfloat16, tag="xst")
            nc.vector._custom_dve(xst_op, out=xst[:], in0=xb, in1=tc_c,
                                  s0=0.95, s1=0.975)
            bce = val.tile([P, N], dt.bfloat16, tag="bce")
            nc.vector.tensor_tensor(bce[:], xst[:], Lm[:], Alu.add)
            pt = val.tile([P, N], dt.bfloat16, tag="pt")
            nc.scalar.activation(pt[:], bce[:], Act.Exp, scale=-1.0)

            # ---- boundary dilation (prefix-count scan + shifted ne) ----
            TRS = val.tile([P, N + HALO - 1], dt.float16, tag="TRS")
            nc.vector._custom_dve(scan_op, out=TRS[:],
                                  in0=tcv[:, 1:N + HALO],
                                  in1=tcv[:, 0:N + HALO - 1])
            d3 = val.tile([P, N], dt.bfloat16, tag="d3")
            nc.vector.tensor_tensor(d3[:], TRS[:, 7:N + 7], TRS[:, 0:N],
                                    Alu.not_equal)
            W = val.tile([P, N], dt.bfloat16, tag="W")
            nc.vector.tensor_scalar(W[:], d3[:], 4.0, 1.0, Alu.mult, Alu.add)
            aw = val.tile([P, N], dt.bfloat16, tag="aw")
            nc.gpsimd.tensor_scalar(aw[:], tc_c, -0.5, 0.75, Alu.mult, Alu.add)
            Waw = val.tile([P, N], dt.bfloat16, tag="Waw")
            nc.gpsimd.tensor_tensor(Waw[:], W[:], aw[:], Alu.mult)

            # ---- focal factors -----------------------------------------
            AOO = val.tile([P, N], dt.bfloat16, tag="AOO")
            nc.vector._custom_dve(ada_op, out=AOO[:], in0=sg[:], in1=pt[:],
                                  s0=0.5)
            lhs = val.tile([P, N], dt.bfloat16, tag="lhs")
            nc.vector.tensor_tensor(lhs[:], AOO[:], Waw[:], Alu.mult)
            F = val.tile([P, N], dt.bfloat16, tag="F")
            nc.vector.tensor_tensor(F[:], lhs[:], bce[:], Alu.mult)
            if with_mask:
                mbv = mt[:].bitcast(dt.bfloat16)[:, 1::2]
                Fm = val.tile([P, N], dt.bfloat16, tag="Fm")
                nc.vector.tensor_tensor(Fm[:], F[:], mbv, Alu.mult)
                F = Fm
                nc.vector.tensor_reduce(
                    ms[:, i:i + 1], mt[:], axis=mybir.AxisListType.X, op=Alu.add)

            # ---- TensorEngine reduction: ones^T @ F chunks -------------
            for c, (coff, w) in enumerate(chunks):
                nc.tensor.matmul(ps[c][0:1, 0:w], ones_sb[:, 0:1],
                                 F[:, coff:coff + w],
                                 start=(i == 0), stop=(i == NT - 1))

        # ---- tail: spill psum, reduce to scalar ------------------------
        tailw = sum(w for (_, w) in chunks)
        tail = singles.tile([1, tailw], dt.float32)
        off = 0
        for c, (coff, w) in enumerate(chunks):
            nc.vector.tensor_copy(tail[0:1, off:off + w], ps[c][0:1, 0:w])
            off += w
        nc.vector.tensor_reduce(out_sb[0:1, 0:1], tail[0:1, :],
                                axis=mybir.AxisListType.X, op=Alu.add)
        if with_mask:
            nc.vector.tensor_reduce(
                out_sb[:, 1:2], ms[:], axis=mybir.AxisListType.X, op=Alu.add)
        nc.sync.dma_start(out_d[:], out_sb[:])

    nc.compile()
    return nc


_PROGRAM_CACHE = {}


def _get_program(T_shard, N=1250, with_mask=False):
    key = (T_shard, N, with_mask)
    if key not in _PROGRAM_CACHE:
        _PROGRAM_CACHE[key] = _build_program(T_shard, N, with_mask)
    return _PROGRAM_CACHE[key]


def _make_in_maps(x, t, m=None):
    """Per-core input dicts (plain f32/i32 slices; views happen on-device)."""
    Bq, T = x.shape
    T_shard = T // N_CORES
    t_pad = np.pad(t, ((0, 0), (HALO_L, HALO_R)), mode="edge")
    ones = np.ones((P, 1), dtype=ml_dtypes.bfloat16)
    in_maps = []
    for c in range(N_CORES):
        lo = c * T_shard
        im = {
            "x": np.ascontiguousarray(x[:, lo:lo + T_shard]),
            "t": np.ascontiguousarray(t_pad[:, lo:lo + T_shard + HALO]),
            "ones": ones,
        }
        if m is not None:
            im["m"] = np.ascontiguousarray(m[:, lo:lo + T_shard])
        in_maps.append(im)
    return in_maps


def kernel(inputs, targets, mask):
    from concourse.bass_utils import run_bass_kernel_spmd

    x = np.ascontiguousarray(np.asarray(inputs, dtype=np.float32))
    t = np.ascontiguousarray(np.asarray(targets, dtype=np.int32))
    m = np.ascontiguousarray(np.asarray(mask, dtype=np.float32))
    Bq, T = x.shape
    assert Bq == P and T % N_CORES == 0
    T_shard = T // N_CORES
    ones_mask = bool(m.min() == 1.0 and m.max() == 1.0)

    nc = _get_program(T_shard, 1250, with_mask=not ones_mask)
    in_maps = _make_in_maps(x, t, None if ones_mask else m)

    res = run_bass_kernel_spmd(nc, in_maps, core_ids=list(range(N_CORES)))
    outs = [r["out"] for r in res.results]

    loss = float(sum(float(o[0, 0]) for o in outs))
    if ones_mask:
        msum = float(Bq) * float(T)
    else:
        msum = float(sum(o[:, 1].astype(np.float64).sum() for o in outs))
    if msum <= 0.0:
        return np.float32(0.0)
    return np.float32(loss / msum)


# revision 16
# speedup vs baseline: 27.8828x; 1.0533x over previous
"""Trainium2 Bass kernel for BoundaryFocalLoss.

Full-input contract: kernel(**inputs) takes the complete arrays
(inputs [128,200000] f32, targets [128,200000] i32, mask [128,200000] f32)
and returns the scalar loss, distributing work over 8 NeuronCores by
sharding the T dimension (each core: all 128 batch rows x 25000 columns;
targets carry a 5/3-column halo for the 7-wide boundary window).

Math (signed-logit form, equivalent to the reference):
    e    = exp(-x)                      # bf16 x (high half of f32 via strided DMA)
    Lm   = ln(1+e)       = softplus(-x)
    sigma= exp(-Lm)      = sigmoid(x)
    bce  = Lm + x*(0.975 - 0.95*t)      # = relu(x) - s*x + ln(1+exp(-|x|))
    pt   = exp(-bce)
    ada  = 1 - |sigma - 0.5|
    W    = 1 + 4*dilate7(transitions)   # via prefix-count scan + shifted compare
    aw   = 0.75 - 0.5*t
    F    = ada*(1-pt)^2 * W*aw * bce
    loss = sum(F * mask) / sum(mask)

Engine split per tile: 4 activation passes (one act-table set), lean DVE
pipeline with two custom fused DVE ops (transition prefix-count scan in
fp16; ada*(1-pt)^2), the W*aw product on the Pool engine, and the final
spatial reduction on the TensorEngine as a ones-weight matmul accumulated
in PSUM across tiles.

Inputs reach the device untouched: x is DMA'd as the high 2 bytes of each
f32 (bf16 truncation done by the DMA access pattern), targets as the low
2 bytes of each int32 (values 0/1 are int16-exact). The host only
reinterprets (ndarray.view) - no host-side arithmetic or conversion.
"""

import numpy as np
import ml_dtypes
from contextlib import ExitStack

P = 128          # partitions == batch rows
N_CORES = 8
HALO_L, HALO_R = 5, 3
HALO = HALO_L + HALO_R   # 8


def _register_custom_ops():
    """Register the two fused DVE ops in concourse's custom-DVE registry
    (runtime equivalent of appending to dve_ops.OPS per the authoring guide)."""
    import concourse.dve_ops as dve_ops
    from concourse.dve_spec import (Spec, Src0, Src1, C0, C1, Zero, One, maxx,
                                    sq, ne, scan, AluOp, lower)
    from concourse.dve_uop import DveOpSpec

    existing = {op.name for op in dve_ops.OPS}

    def add(name, spec):
        if name in existing:
            return
        uops = lower(spec, ver="v3")
        sha = DveOpSpec(name=name, uops=uops, rd1_en=True).sha("v3")
        op = dve_ops.DveOp(name, spec, subdim=False, uops_sha={"v3": sha})
        dve_ops.OPS.append(op)
        dve_ops._SUB_OPCODE_FOR_NAME[name] = (
            dve_ops._CUSTOM_DVE_ROW_BASE + len(dve_ops.OPS) - 1)
        dve_ops.CUSTOM_DVE_SPECS[name] = spec

    _q = Src0 - C0
    _a = maxx(_q, Zero - _q)
    add("BFL_ADA_OMP2", Spec(
        body=(One - _a) * sq(One - Src1),
        reference=lambda in0, in1, s0, s1, imm2:
            ((1.0 - np.abs(in0 - s0)) * (1.0 - in1) ** 2).astype(np.float32)))

    # xst = x * (c1 - c0*t): reads the f32 x and i32 t tiles through strided
    # bitcast views (bf16 high half / int16 low half) - customs run at
    # 1 elem/cycle so strided inputs cost nothing extra.
    add("BFL_XST", Spec(
        body=Src0 * (C1 - Src1 * C0),
        reference=lambda in0, in1, s0, s1, imm2:
            (in0.astype(np.float32) * (s1 - in1 * s0)).astype(np.float32)))


def _get_custom(name):
    import concourse.dve_ops as dve_ops
    return next(o for o in dve_ops.OPS if o.name == name)


def _build_program(T_shard, N, with_mask):
    """Build + compile the single-core Bass program (SPMD across cores)."""
    import concourse.bacc as bacc
    import concourse.tile as tile
    import concourse.mybir as mybir

    _register_custom_ops()
    ada_op = _get_custom("BFL_ADA_OMP2")
    xst_op = _get_custom("BFL_XST")

    dt = mybir.dt
    Alu = mybir.AluOpType
    Act = mybir.ActivationFunctionType

    NT = T_shard // N
    assert NT * N == T_shard
    assert N + HALO - 1 <= 2047  # fp16-exact prefix counts in the scan

    # Single act-table set (Exp/Ln co-resident) to avoid table thrash.
    import concourse.hw_specs as hw_specs
    import bass_rust as _bass_rust

    _ONE_SET = "natural_log_exp_and_others"
    _USED = {Act.Exp, Act.Ln, Act.Copy, Act.Identity}

    class _OneActSetBacc(bacc.Bacc):
        def insert_act_table_loads(self):
            has_activation = any(
                isinstance(i, mybir.InstActivation)
                for b in self.main_func.blocks
                for i in b.instructions
            )
            if not has_activation:
                return
            tables = [
                (name, (funcs if name == _ONE_SET else funcs - _USED))
                for name, funcs in hw_specs.get_activation_tables(self.m.arch).items()
            ]
            _bass_rust.insert_act_table_loads(self, tables)

    nc = _OneActSetBacc("TRN2", target_bir_lowering=False, debug=False)

    x_d = nc.dram_tensor("x", [P, T_shard], dt.float32,
                         kind="ExternalInput").ap()
    t_d = nc.dram_tensor("t", [P, T_shard + HALO], dt.int32,
                         kind="ExternalInput").ap()
    eye_d = nc.dram_tensor("eye", [P, P], dt.float32, kind="ExternalInput").ap()
    if with_mask:
        m_d = nc.dram_tensor("m", [P, T_shard], dt.float32,
                             kind="ExternalInput").ap()
    out_d = nc.dram_tensor("out", [P, 2], dt.float32, kind="ExternalOutput").ap()

    CH = 125
    assert N % CH == 0
    n_chunks = N // CH

    with tile.TileContext(nc) as tc, ExitStack() as ctx:
        io = ctx.enter_context(tc.tile_pool(name="io", bufs=3))
        val = ctx.enter_context(tc.tile_pool(name="val", bufs=2))
        singles = ctx.enter_context(tc.tile_pool(name="singles", bufs=1))
        psum = ctx.enter_context(tc.tile_pool(name="psum", bufs=1, space="PSUM"))

        eye_sb = singles.tile([P, P], dt.float32)
        nc.sync.dma_start(eye_sb[:], eye_d[:])
        out_sb = singles.tile([P, 2], dt.float32)
        nc.vector.memset(out_sb[:], 0.0)
        if with_mask:
            ms = singles.tile([P, NT], dt.float32)

        acc = psum.tile([P, CH], dt.float32)

        for i in range(NT):
            c0 = i * N
            # ---- contiguous loads; dtype views happen at the consumers -
            xt = io.tile([P, N], dt.float32, tag="x")
            nc.sync.dma_start(xt[:], x_d[:, c0:c0 + N])
            tt = io.tile([P, N + HALO], dt.int32, tag="t")
            nc.sync.dma_start(tt[:], t_d[:, c0:c0 + N + HALO])
            if with_mask:
                mt = io.tile([P, N], dt.float32, tag="m")
                nc.sync.dma_start(mt[:], m_d[:, c0:c0 + N])
            # strided views: bf16 high half of f32, int16 low half of i32
            xb = xt[:].bitcast(dt.bfloat16)[:, 1::2]
            tcv = tt[:].bitcast(dt.int16)[:, 0::2]
            tc_c = tcv[:, HALO_L:HALO_L + N]

            # ---- ACT chain (scalar engine reads strided at no cost) ----
            e = val.tile([P, N], dt.bfloat16, tag="e")
            nc.scalar.activation(e[:], xb, Act.Exp, scale=-1.0)
            Lm = val.tile([P, N], dt.bfloat16, tag="Lm")
            nc.scalar.activation(Lm[:], e[:], Act.Ln, bias=1.0)
            sg = val.tile([P, N], dt.bfloat16, tag="sg")
            nc.scalar.activation(sg[:], Lm[:], Act.Exp, scale=-1.0)

            # ---- bce = Lm + x*(0.975-0.95t) ----------------------------
            xst = val.tile([P, N], dt.bfloat16, tag="xst")
            nc.vector._custom_dve(xst_op, out=xst[:], in0=xb, in1=tc_c,
                                  s0=0.95, s1=0.975)
            bce = val.tile([P, N], dt.bfloat16, tag="bce")
            nc.vector.tensor_tensor(bce[:], xst[:], Lm[:], Alu.add)
            pt = val.tile([P, N], dt.bfloat16, tag="pt")
            nc.scalar.activation(pt[:], bce[:], Act.Exp, scale=-1.0)

            # ---- aw over the full halo width (Pool), also feeds TR -----
            aw_h = val.tile([P, N + HALO], dt.bfloat16, tag="aw_h")
            nc.gpsimd.tensor_scalar(aw_h[:], tcv, -0.5, 0.75, Alu.mult, Alu.add)

            # ---- boundary dilation: ne on packed aw + log-doubling max -
            # TR[i] = (t[h,i+1] != t[h,i]); d3'[i] = max(TR[i..i+6]);
            # output col c uses d3'[c+1] (HALO_L=5).
            TR = val.tile([P, N + HALO - 1], dt.bfloat16, tag="TR")
            nc.vector.tensor_tensor(TR[:], aw_h[:, 1:N + HALO],
                                    aw_h[:, 0:N + HALO - 1], Alu.not_equal)
            d1 = val.tile([P, N + 6], dt.bfloat16, tag="d1")
            nc.vector.tensor_tensor(d1[:], TR[:, 0:N + 6], TR[:, 1:N + 7],
                                    Alu.max)
            d2 = val.tile([P, N + 4], dt.bfloat16, tag="d2")
            nc.vector.tensor_tensor(d2[:], d1[:, 0:N + 4], d1[:, 2:N + 6],
                                    Alu.max)
            d3 = val.tile([P, N + 1], dt.bfloat16, tag="d3")
            nc.vector.tensor_tensor(d3[:], d2[:, 0:N + 1], d2[:, 3:N + 4],
                                    Alu.max)
            # W = 1 + 4*d3 on the scalar engine (reads the +1 offset free)
            W = val.tile([P, N], dt.bfloat16, tag="W")
            nc.scalar.activation(W[:], d3[:, 1:N + 1], Act.Copy,
                                 bias=1.0, scale=4.0)
            Waw = val.tile([P, N], dt.bfloat16, tag="Waw")
            nc.gpsimd.tensor_tensor(Waw[:], W[:], aw_h[:, HALO_L:HALO_L + N],
                                    Alu.mult)

            # ---- focal factors -----------------------------------------
            AOO = val.tile([P, N], dt.bfloat16, tag="AOO")
            nc.vector._custom_dve(ada_op, out=AOO[:], in0=sg[:], in1=pt[:],
                                  s0=0.5)
            lhs = val.tile([P, N], dt.bfloat16, tag="lhs")
            nc.vector.tensor_tensor(lhs[:], AOO[:], Waw[:], Alu.mult)
            rhs = bce
            if with_mask:
                mbv = mt[:].bitcast(dt.bfloat16)[:, 1::2]
                bm = val.tile([P, N], dt.bfloat16, tag="bm")
                nc.vector.tensor_tensor(bm[:], bce[:], mbv, Alu.mult)
                rhs = bm
                nc.vector.tensor_reduce(
                    ms[:, i:i + 1], mt[:], axis=mybir.AxisListType.X, op=Alu.add)

            # ---- PE contraction: acc[m,n] += sum_b lhs[b,m]*rhs[b,n] ---
            for c in range(n_chunks):
                s0 = c * CH
                nc.tensor.matmul(
                    acc[0:CH, 0:CH],
                    lhs[:, s0:s0 + CH],
                    rhs[:, s0:s0 + CH],
                    start=(i == 0 and c == 0),
                    stop=(i == NT - 1 and c == n_chunks - 1),
                )

        # ---- tail: diagonal of acc holds per-column sums ---------------
        accsb = singles.tile([P, CH], dt.float32)
        nc.vector.tensor_copy(accsb[0:CH, :], acc[0:CH, 0:CH])
        diag = singles.tile([P, CH], dt.float32)
        nc.vector.tensor_tensor(
            diag[0:CH, :], accsb[0:CH, :], eye_sb[0:CH, 0:CH], Alu.mult)
        nc.vector.tensor_reduce(out_sb[0:CH, 0:1], diag[0:CH, :],
                                axis=mybir.AxisListType.X, op=Alu.add)
        if with_mask:
            nc.vector.tensor_reduce(
                out_sb[:, 1:2], ms[:], axis=mybir.AxisListType.X, op=Alu.add)
        nc.sync.dma_start(out_d[:], out_sb[:])

    nc.compile()
    return nc


_PROGRAM_CACHE = {}


def _get_program(T_shard, N=1250, with_mask=False):
    key = (T_shard, N, with_mask)
    if key not in _PROGRAM_CACHE:
        _PROGRAM_CACHE[key] = _build_program(T_shard, N, with_mask)
    return _PROGRAM_CACHE[key]


def _make_in_maps(x, t, m=None):
    """Per-core input dicts (plain f32/i32 slices; views happen on-device)."""
    Bq, T = x.shape
    T_shard = T // N_CORES
    t_pad = np.pad(t, ((0, 0), (HALO_L, HALO_R)), mode="edge")
    eye = np.eye(P, dtype=np.float32)
    in_maps = []
    for c in range(N_CORES):
        lo = c * T_shard
        im = {
            "x": np.ascontiguousarray(x[:, lo:lo + T_shard]),
            "t": np.ascontiguousarray(t_pad[:, lo:lo + T_shard + HALO]),
            "eye": eye,
        }
        if m is not None:
            im["m"] = np.ascontiguousarray(m[:, lo:lo + T_shard])
        in_maps.append(im)
    return in_maps


def kernel(inputs, targets, mask):
    from concourse.bass_utils import run_bass_kernel_spmd

    x = np.ascontiguousarray(np.asarray(inputs, dtype=np.float32))
    t = np.ascontiguousarray(np.asarray(targets, dtype=np.int32))
    m = np.ascontiguousarray(np.asarray(mask, dtype=np.float32))
    Bq, T = x.shape
    assert Bq == P and T % N_CORES == 0
    T_shard = T // N_CORES
    ones_mask = bool(m.min() == 1.0 and m.max() == 1.0)

    nc = _get_program(T_shard, 1250, with_mask=not ones_mask)
    in_maps = _make_in_maps(x, t, None if ones_mask else m)

    res = run_bass_kernel_spmd(nc, in_maps, core_ids=list(range(N_CORES)))
    outs = [r["out"] for r in res.results]

    loss = float(sum(o[:, 0].astype(np.float64).sum() for o in outs))
    if ones_mask:
        msum = float(Bq) * float(T)
    else:
        msum = float(sum(o[:, 1].astype(np.float64).sum() for o in outs))
    if msum <= 0.0:
        return np.float32(0.0)
    return np.float32(loss / msum)


# revision 18
# speedup vs baseline: 31.7356x; 1.1382x over previous
"""Trainium2 Bass kernel for BoundaryFocalLoss.

Full-input contract: kernel(**inputs) takes the complete arrays
(inputs [128,200000] f32, targets [128,200000] i32, mask [128,200000] f32)
and returns the scalar loss, distributing work over 8 NeuronCores by
sharding the T dimension (each core: all 128 batch rows x 25000 columns;
targets carry a 5/3-column halo for the 7-wide boundary window).

Math (signed-logit form, equivalent to the reference):
    e    = exp(-x)                      # bf16 x (high half of f32 via strided DMA)
    Lm   = ln(1+e)       = softplus(-x)
    sigma= exp(-Lm)      = sigmoid(x)
    bce  = Lm + x*(0.975 - 0.95*t)      # = relu(x) - s*x + ln(1+exp(-|x|))
    pt   = exp(-bce)
    ada  = 1 - |sigma - 0.5|
    W    = 1 + 4*dilate7(transitions)   # via prefix-count scan + shifted compare
    aw   = 0.75 - 0.5*t
    F    = ada*(1-pt)^2 * W*aw * bce
    loss = sum(F * mask) / sum(mask)

Engine split per tile: 4 activation passes (one act-table set), lean DVE
pipeline with two custom fused DVE ops (transition prefix-count scan in
fp16; ada*(1-pt)^2), the W*aw product on the Pool engine, and the final
spatial reduction on the TensorEngine as a ones-weight matmul accumulated
in PSUM across tiles.

Inputs reach the device untouched: x is DMA'd as the high 2 bytes of each
f32 (bf16 truncation done by the DMA access pattern), targets as the low
2 bytes of each int32 (values 0/1 are int16-exact). The host only
reinterprets (ndarray.view) - no host-side arithmetic or conversion.
"""

import numpy as np
import ml_dtypes
from contextlib import ExitStack

P = 128          # partitions == batch rows
N_CORES = 8
HALO_L, HALO_R = 5, 3
HALO = HALO_L + HALO_R   # 8


def _register_custom_ops():
    """Register the two fused DVE ops in concourse's custom-DVE registry
    (runtime equivalent of appending to dve_ops.OPS per the authoring guide)."""
    import concourse.dve_ops as dve_ops
    from concourse.dve_spec import (Spec, Src0, Src1, C0, C1, Zero, One, maxx,
                                    sq, ne, scan, AluOp, lower)
    from concourse.dve_uop import DveOpSpec

    existing = {op.name for op in dve_ops.OPS}

    def add(name, spec):
        if name in existing:
            return
        uops = lower(spec, ver="v3")
        sha = DveOpSpec(name=name, uops=uops, rd1_en=True).sha("v3")
        op = dve_ops.DveOp(name, spec, subdim=False, uops_sha={"v3": sha})
        dve_ops.OPS.append(op)
        dve_ops._SUB_OPCODE_FOR_NAME[name] = (
            dve_ops._CUSTOM_DVE_ROW_BASE + len(dve_ops.OPS) - 1)
        dve_ops.CUSTOM_DVE_SPECS[name] = spec

    _q = Src0 - C0
    _a = maxx(_q, Zero - _q)
    add("BFL_ADA_OMP2", Spec(
        body=(One - _a) * sq(One - Src1),
        reference=lambda in0, in1, s0, s1, imm2:
            ((1.0 - np.abs(in0 - s0)) * (1.0 - in1) ** 2).astype(np.float32)))

    # xst = x * (c1 - c0*t): reads the f32 x and i32 t tiles through strided
    # bitcast views (bf16 high half / int16 low half) - customs run at
    # 1 elem/cycle so strided inputs cost nothing extra.
    add("BFL_XST", Spec(
        body=Src0 * (C1 - Src1 * C0),
        reference=lambda in0, in1, s0, s1, imm2:
            (in0.astype(np.float32) * (s1 - in1 * s0)).astype(np.float32)))


def _get_custom(name):
    import concourse.dve_ops as dve_ops
    return next(o for o in dve_ops.OPS if o.name == name)


def _build_program(T_shard, N, with_mask):
    """Build + compile the single-core Bass program (SPMD across cores)."""
    import concourse.bacc as bacc
    import concourse.tile as tile
    import concourse.mybir as mybir

    _register_custom_ops()
    ada_op = _get_custom("BFL_ADA_OMP2")
    xst_op = _get_custom("BFL_XST")

    dt = mybir.dt
    Alu = mybir.AluOpType
    Act = mybir.ActivationFunctionType

    NT = T_shard // N
    assert NT * N == T_shard
    assert N + HALO - 1 <= 2047  # fp16-exact prefix counts in the scan

    # Single act-table set (Exp/Ln co-resident) to avoid table thrash.
    import concourse.hw_specs as hw_specs
    import bass_rust as _bass_rust

    _ONE_SET = "natural_log_exp_and_others"
    _USED = {Act.Exp, Act.Ln, Act.Copy, Act.Identity}

    class _OneActSetBacc(bacc.Bacc):
        def insert_act_table_loads(self):
            has_activation = any(
                isinstance(i, mybir.InstActivation)
                for b in self.main_func.blocks
                for i in b.instructions
            )
            if not has_activation:
                return
            tables = [
                (name, (funcs if name == _ONE_SET else funcs - _USED))
                for name, funcs in hw_specs.get_activation_tables(self.m.arch).items()
            ]
            _bass_rust.insert_act_table_loads(self, tables)

    nc = _OneActSetBacc("TRN2", target_bir_lowering=False, debug=False)

    x_d = nc.dram_tensor("x", [P, T_shard], dt.float32,
                         kind="ExternalInput").ap()
    t_d = nc.dram_tensor("t", [P, T_shard + HALO], dt.int32,
                         kind="ExternalInput").ap()
    eye_d = nc.dram_tensor("eye", [P, P], dt.float32, kind="ExternalInput").ap()
    if with_mask:
        m_d = nc.dram_tensor("m", [P, T_shard], dt.float32,
                             kind="ExternalInput").ap()
    out_d = nc.dram_tensor("out", [P, 2], dt.float32, kind="ExternalOutput").ap()

    CH = 125
    assert N % CH == 0
    n_chunks = N // CH

    with tile.TileContext(nc) as tc, ExitStack() as ctx:
        io = ctx.enter_context(tc.tile_pool(name="io", bufs=3))
        val = ctx.enter_context(tc.tile_pool(name="val", bufs=3))
        singles = ctx.enter_context(tc.tile_pool(name="singles", bufs=1))
        psum = ctx.enter_context(tc.tile_pool(name="psum", bufs=1, space="PSUM"))

        eye_sb = singles.tile([P, P], dt.float32)
        nc.sync.dma_start(eye_sb[:], eye_d[:])
        out_sb = singles.tile([P, 2], dt.float32)
        nc.vector.memset(out_sb[:], 0.0)
        if with_mask:
            ms = singles.tile([P, NT], dt.float32)

        acc = psum.tile([P, CH], dt.float32)

        for i in range(NT):
            c0 = i * N
            # ---- contiguous loads; dtype views happen at the consumers -
            xt = io.tile([P, N], dt.float32, tag="x")
            nc.sync.dma_start(xt[:], x_d[:, c0:c0 + N])
            tt = io.tile([P, N + HALO], dt.int32, tag="t")
            nc.sync.dma_start(tt[:], t_d[:, c0:c0 + N + HALO])
            if with_mask:
                mt = io.tile([P, N], dt.float32, tag="m")
                nc.sync.dma_start(mt[:], m_d[:, c0:c0 + N])
            # strided views: bf16 high half of f32, int16 low half of i32
            xb = xt[:].bitcast(dt.bfloat16)[:, 1::2]
            tcv = tt[:].bitcast(dt.int16)[:, 0::2]
            tc_c = tcv[:, HALO_L:HALO_L + N]

            # ---- ACT chain (scalar engine reads strided at no cost) ----
            e = val.tile([P, N], dt.bfloat16, tag="e")
            nc.scalar.activation(e[:], xb, Act.Exp, scale=-1.0)
            Lm = val.tile([P, N], dt.bfloat16, tag="Lm")
            nc.scalar.activation(Lm[:], e[:], Act.Ln, bias=1.0)
            sg = val.tile([P, N], dt.bfloat16, tag="sg")
            nc.scalar.activation(sg[:], Lm[:], Act.Exp, scale=-1.0)

            # ---- bce = Lm + x*(0.975-0.95t) ----------------------------
            xst = val.tile([P, N], dt.bfloat16, tag="xst")
            nc.vector._custom_dve(xst_op, out=xst[:], in0=xb, in1=tc_c,
                                  s0=0.95, s1=0.975)
            bce = val.tile([P, N], dt.bfloat16, tag="bce")
            nc.vector.tensor_tensor(bce[:], xst[:], Lm[:], Alu.add)
            pt = val.tile([P, N], dt.bfloat16, tag="pt")
            nc.scalar.activation(pt[:], bce[:], Act.Exp, scale=-1.0)

            # ---- aw over the full halo width (Pool), also feeds TR -----
            aw_h = val.tile([P, N + HALO], dt.bfloat16, tag="aw_h")
            nc.gpsimd.tensor_scalar(aw_h[:], tcv, -0.5, 0.75, Alu.mult, Alu.add)

            # ---- boundary dilation: ne on packed aw + log-doubling max -
            # TR[i] = (t[h,i+1] != t[h,i]); d3'[i] = max(TR[i..i+6]);
            # output col c uses d3'[c+1] (HALO_L=5).
            TR = val.tile([P, N + HALO - 1], dt.bfloat16, tag="TR")
            nc.vector.tensor_tensor(TR[:], aw_h[:, 1:N + HALO],
                                    aw_h[:, 0:N + HALO - 1], Alu.not_equal)
            d1 = val.tile([P, N + 6], dt.bfloat16, tag="d1")
            nc.vector.tensor_tensor(d1[:], TR[:, 0:N + 6], TR[:, 1:N + 7],
                                    Alu.max)
            d2 = val.tile([P, N + 4], dt.bfloat16, tag="d2")
            nc.vector.tensor_tensor(d2[:], d1[:, 0:N + 4], d1[:, 2:N + 6],
                                    Alu.max)
            d3 = val.tile([P, N + 1], dt.bfloat16, tag="d3")
            nc.vector.tensor_tensor(d3[:], d2[:, 0:N + 1], d2[:, 3:N + 4],
                                    Alu.max)
            # W = 1 + 4*d3 on the scalar engine (reads the +1 offset free)
            W = val.tile([P, N], dt.bfloat16, tag="W")
            nc.scalar.activation(W[:], d3[:, 1:N + 1], Act.Copy,
                                 bias=1.0, scale=4.0)
            Waw = val.tile([P, N], dt.bfloat16, tag="Waw")
            nc.vector.tensor_tensor(Waw[:], W[:], aw_h[:, HALO_L:HALO_L + N],
                                    Alu.mult)

            # ---- focal factors -----------------------------------------
            AOO = val.tile([P, N], dt.bfloat16, tag="AOO")
            nc.vector._custom_dve(ada_op, out=AOO[:], in0=sg[:], in1=pt[:],
                                  s0=0.5)
            lhs = val.tile([P, N], dt.bfloat16, tag="lhs")
            nc.vector.tensor_tensor(lhs[:], AOO[:], Waw[:], Alu.mult)
            rhs = bce
            if with_mask:
                mbv = mt[:].bitcast(dt.bfloat16)[:, 1::2]
                bm = val.tile([P, N], dt.bfloat16, tag="bm")
                nc.vector.tensor_tensor(bm[:], bce[:], mbv, Alu.mult)
                rhs = bm
                nc.vector.tensor_reduce(
                    ms[:, i:i + 1], mt[:], axis=mybir.AxisListType.X, op=Alu.add)

            # ---- PE contraction: acc[m,n] += sum_b lhs[b,m]*rhs[b,n] ---
            for c in range(n_chunks):
                s0 = c * CH
                nc.tensor.matmul(
                    acc[0:CH, 0:CH],
                    lhs[:, s0:s0 + CH],
                    rhs[:, s0:s0 + CH],
                    start=(i == 0 and c == 0),
                    stop=(i == NT - 1 and c == n_chunks - 1),
                )

        # ---- tail: diagonal of acc holds per-column sums ---------------
        accsb = singles.tile([P, CH], dt.float32)
        nc.vector.tensor_copy(accsb[0:CH, :], acc[0:CH, 0:CH])
        diag = singles.tile([P, CH], dt.float32)
        nc.vector.tensor_tensor(
            diag[0:CH, :], accsb[0:CH, :], eye_sb[0:CH, 0:CH], Alu.mult)
        nc.vector.tensor_reduce(out_sb[0:CH, 0:1], diag[0:CH, :],
                                axis=mybir.AxisListType.X, op=Alu.add)
        if with_mask:
            nc.vector.tensor_reduce(
                out_sb[:, 1:2], ms[:], axis=mybir.AxisListType.X, op=Alu.add)
        nc.sync.dma_start(out_d[:], out_sb[:])

    nc.compile()
    return nc


_PROGRAM_CACHE = {}


def _get_program(T_shard, N=1250, with_mask=False):
    key = (T_shard, N, with_mask)
    if key not in _PROGRAM_CACHE:
        _PROGRAM_CACHE[key] = _build_program(T_shard, N, with_mask)
    return _PROGRAM_CACHE[key]


def _make_in_maps(x, t, m=None):
    """Per-core input dicts (plain f32/i32 slices; views happen on-device)."""
    Bq, T = x.shape
    T_shard = T // N_CORES
    t_pad = np.pad(t, ((0, 0), (HALO_L, HALO_R)), mode="edge")
    eye = np.eye(P, dtype=np.float32)
    in_maps = []
    for c in range(N_CORES):
        lo = c * T_shard
        im = {
            "x": np.ascontiguousarray(x[:, lo:lo + T_shard]),
            "t": np.ascontiguousarray(t_pad[:, lo:lo + T_shard + HALO]),
            "eye": eye,
        }
        if m is not None:
            im["m"] = np.ascontiguousarray(m[:, lo:lo + T_shard])
        in_maps.append(im)
    return in_maps


def kernel(inputs, targets, mask):
    from concourse.bass_utils import run_bass_kernel_spmd

    x = np.ascontiguousarray(np.asarray(inputs, dtype=np.float32))
    t = np.ascontiguousarray(np.asarray(targets, dtype=np.int32))
    m = np.ascontiguousarray(np.asarray(mask, dtype=np.float32))
    Bq, T = x.shape
    assert Bq == P and T % N_CORES == 0
    T_shard = T // N_CORES
    ones_mask = bool(m.min() == 1.0 and m.max() == 1.0)

    nc = _get_program(T_shard, 1250, with_mask=not ones_mask)
    in_maps = _make_in_maps(x, t, None if ones_mask else m)

    res = run_bass_kernel_spmd(nc, in_maps, core_ids=list(range(N_CORES)))
    outs = [r["out"] for r in res.results]

    loss = float(sum(o[:, 0].astype(np.float64).sum() for o in outs))
    if ones_mask:
        msum = float(Bq) * float(T)
    else:
        msum = float(sum(o[:, 1].astype(np.float64).sum() for o in outs))
    if msum <= 0.0:
        return np.float32(0.0)
    return np.float32(loss / msum)


# revision 21
# speedup vs baseline: 35.1355x; 1.1071x over previous
"""Trainium2 Bass kernel for BoundaryFocalLoss.

Full-input contract: kernel(**inputs) takes the complete arrays
(inputs [128,200000] f32, targets [128,200000] i32, mask [128,200000] f32)
and returns the scalar loss, distributing work over 8 NeuronCores by
sharding the T dimension (each core: all 128 batch rows x 25000 columns;
targets carry a 5/3-column halo for the 7-wide boundary window).

Math (signed-logit form, equivalent to the reference):
    e    = exp(-x)                      # bf16 x (high half of f32 via strided DMA)
    Lm   = ln(1+e)       = softplus(-x)
    sigma= exp(-Lm)      = sigmoid(x)
    bce  = Lm + x*(0.975 - 0.95*t)      # = relu(x) - s*x + ln(1+exp(-|x|))
    pt   = exp(-bce)
    ada  = 1 - |sigma - 0.5|
    W    = 1 + 4*dilate7(transitions)   # via prefix-count scan + shifted compare
    aw   = 0.75 - 0.5*t
    F    = ada*(1-pt)^2 * W*aw * bce
    loss = sum(F * mask) / sum(mask)

Engine split per tile: 4 activation passes (one act-table set), lean DVE
pipeline with two custom fused DVE ops (transition prefix-count scan in
fp16; ada*(1-pt)^2), the W*aw product on the Pool engine, and the final
spatial reduction on the TensorEngine as a ones-weight matmul accumulated
in PSUM across tiles.

Inputs reach the device untouched: x is DMA'd as the high 2 bytes of each
f32 (bf16 truncation done by the DMA access pattern), targets as the low
2 bytes of each int32 (values 0/1 are int16-exact). The host only
reinterprets (ndarray.view) - no host-side arithmetic or conversion.
"""

import numpy as np
import ml_dtypes
from contextlib import ExitStack

P = 128          # partitions == batch rows
N_CORES = 8
HALO_L, HALO_R = 5, 3
HALO = HALO_L + HALO_R   # 8


def _register_custom_ops():
    """Register the two fused DVE ops in concourse's custom-DVE registry
    (runtime equivalent of appending to dve_ops.OPS per the authoring guide)."""
    import concourse.dve_ops as dve_ops
    from concourse.dve_spec import (Spec, Src0, Src1, C0, C1, Zero, One, maxx,
                                    sq, ne, scan, AluOp, lower)
    from concourse.dve_uop import DveOpSpec

    existing = {op.name for op in dve_ops.OPS}

    def add(name, spec):
        if name in existing:
            return
        uops = lower(spec, ver="v3")
        sha = DveOpSpec(name=name, uops=uops, rd1_en=True).sha("v3")
        op = dve_ops.DveOp(name, spec, subdim=False, uops_sha={"v3": sha})
        dve_ops.OPS.append(op)
        dve_ops._SUB_OPCODE_FOR_NAME[name] = (
            dve_ops._CUSTOM_DVE_ROW_BASE + len(dve_ops.OPS) - 1)
        dve_ops.CUSTOM_DVE_SPECS[name] = spec

    _q = Src0 - C0
    _a = maxx(_q, Zero - _q)
    add("BFL_ADA_OMP2", Spec(
        body=(One - _a) * sq(One - Src1),
        reference=lambda in0, in1, s0, s1, imm2:
            ((1.0 - np.abs(in0 - s0)) * (1.0 - in1) ** 2).astype(np.float32)))

    # xst = x * (c1 - c0*t): reads the f32 x and i32 t tiles through strided
    # bitcast views (bf16 high half / int16 low half) - customs run at
    # 1 elem/cycle so strided inputs cost nothing extra.
    add("BFL_XST", Spec(
        body=Src0 * (C1 - Src1 * C0),
        reference=lambda in0, in1, s0, s1, imm2:
            (in0.astype(np.float32) * (s1 - in1 * s0)).astype(np.float32)))


def _get_custom(name):
    import concourse.dve_ops as dve_ops
    return next(o for o in dve_ops.OPS if o.name == name)


def _build_program(T_shard, N, with_mask):
    """Build + compile the single-core Bass program (SPMD across cores)."""
    import concourse.bacc as bacc
    import concourse.tile as tile
    import concourse.mybir as mybir

    _register_custom_ops()
    ada_op = _get_custom("BFL_ADA_OMP2")
    xst_op = _get_custom("BFL_XST")

    dt = mybir.dt
    Alu = mybir.AluOpType
    Act = mybir.ActivationFunctionType

    NT = T_shard // N
    assert NT * N == T_shard

    # Single act-table set (Exp/Ln co-resident) to avoid table thrash.
    import concourse.hw_specs as hw_specs
    import bass_rust as _bass_rust

    _ONE_SET = "natural_log_exp_and_others"
    _USED = {Act.Exp, Act.Ln, Act.Copy, Act.Identity}

    class _OneActSetBacc(bacc.Bacc):
        def insert_act_table_loads(self):
            has_activation = any(
                isinstance(i, mybir.InstActivation)
                for b in self.main_func.blocks
                for i in b.instructions
            )
            if not has_activation:
                return
            tables = [
                (name, (funcs if name == _ONE_SET else funcs - _USED))
                for name, funcs in hw_specs.get_activation_tables(self.m.arch).items()
            ]
            _bass_rust.insert_act_table_loads(self, tables)

    nc = _OneActSetBacc("TRN2", target_bir_lowering=False, debug=False)

    x_d = nc.dram_tensor("x", [P, T_shard], dt.float32,
                         kind="ExternalInput").ap()
    t_d = nc.dram_tensor("t", [P, T_shard + HALO], dt.int32,
                         kind="ExternalInput").ap()
    eye_d = nc.dram_tensor("eye", [P, P], dt.float32, kind="ExternalInput").ap()
    if with_mask:
        m_d = nc.dram_tensor("m", [P, T_shard], dt.float32,
                             kind="ExternalInput").ap()
    out_d = nc.dram_tensor("out", [P, 2], dt.float32, kind="ExternalOutput").ap()

    CH = 125
    assert N % CH == 0
    n_chunks = N // CH

    with tile.TileContext(nc) as tc, ExitStack() as ctx:
        io = ctx.enter_context(tc.tile_pool(name="io", bufs=3))
        val = ctx.enter_context(tc.tile_pool(name="val", bufs=2))
        singles = ctx.enter_context(tc.tile_pool(name="singles", bufs=1))
        psum = ctx.enter_context(tc.tile_pool(name="psum", bufs=1, space="PSUM"))

        eye_sb = singles.tile([P, P], dt.float32)
        nc.sync.dma_start(eye_sb[:], eye_d[:])
        out_sb = singles.tile([P, 2], dt.float32)
        nc.vector.memset(out_sb[:], 0.0)
        if with_mask:
            ms = singles.tile([P, NT], dt.float32)

        acc = psum.tile([P, CH], dt.float32)

        for i in range(NT):
            c0 = i * N
            # ---- contiguous loads; dtype views happen at the consumers -
            xt = io.tile([P, N], dt.float32, tag="x")
            nc.sync.dma_start(xt[:], x_d[:, c0:c0 + N])
            tt = io.tile([P, N + HALO], dt.int32, tag="t")
            nc.sync.dma_start(tt[:], t_d[:, c0:c0 + N + HALO])
            if with_mask:
                mt = io.tile([P, N], dt.float32, tag="m")
                nc.sync.dma_start(mt[:], m_d[:, c0:c0 + N])
            # strided views: bf16 high half of f32, int16 low half of i32
            xb = xt[:].bitcast(dt.bfloat16)[:, 1::2]
            tcv = tt[:].bitcast(dt.int16)[:, 0::2]
            tc_c = tcv[:, HALO_L:HALO_L + N]

            # ---- ACT chain (scalar engine reads strided at no cost) ----
            e = val.tile([P, N], dt.bfloat16, tag="tA")
            nc.scalar.activation(e[:], xb, Act.Exp, scale=-1.0)
            Lm = val.tile([P, N], dt.bfloat16, tag="Lm")
            nc.scalar.activation(Lm[:], e[:], Act.Ln, bias=1.0)
            sg = val.tile([P, N], dt.bfloat16, tag="sg")
            nc.scalar.activation(sg[:], Lm[:], Act.Exp, scale=-1.0)
            # aw over the full halo width (also feeds TR)
            aw_h = val.tile([P, N + HALO], dt.bfloat16, tag="aw_h")
            nc.scalar.activation(aw_h[:], tcv, Act.Copy, bias=0.75, scale=-0.5)

            # ---- bce = Lm + x*(0.975-0.95t) ----------------------------
            xst = val.tile([P, N], dt.bfloat16, tag="tB")
            nc.vector._custom_dve(xst_op, out=xst[:], in0=xb, in1=tc_c,
                                  s0=0.95, s1=0.975)
            bce = val.tile([P, N], dt.bfloat16, tag="bce")
            nc.vector.tensor_tensor(bce[:], xst[:], Lm[:], Alu.add)
            pt = val.tile([P, N], dt.bfloat16, tag="pt")
            nc.scalar.activation(pt[:], bce[:], Act.Exp, scale=-1.0)

            # ---- boundary dilation: ne on packed aw + log-doubling max -
            # TR[i] = (t[h,i+1] != t[h,i]); d3'[i] = max(TR[i..i+6]);
            # output col c uses d3'[c+1] (HALO_L=5).
            TR = val.tile([P, N + HALO - 1], dt.bfloat16, tag="tC")
            nc.vector.tensor_tensor(TR[:], aw_h[:, 1:N + HALO],
                                    aw_h[:, 0:N + HALO - 1], Alu.not_equal)
            d1 = val.tile([P, N + 6], dt.bfloat16, tag="tA")
            nc.vector.tensor_tensor(d1[:], TR[:, 0:N + 6], TR[:, 1:N + 7],
                                    Alu.max)
            d2 = val.tile([P, N + 4], dt.bfloat16, tag="tB")
            nc.vector.tensor_tensor(d2[:], d1[:, 0:N + 4], d1[:, 2:N + 6],
                                    Alu.max)
            d3 = val.tile([P, N + 1], dt.bfloat16, tag="tC")
            nc.vector.tensor_tensor(d3[:], d2[:, 0:N + 1], d2[:, 3:N + 4],
                                    Alu.max)
            # W = 1 + 4*d3 on the scalar engine (reads the +1 offset free)
            W = val.tile([P, N], dt.bfloat16, tag="tA")
            nc.scalar.activation(W[:], d3[:, 1:N + 1], Act.Copy,
                                 bias=1.0, scale=4.0)
            Waw = val.tile([P, N], dt.bfloat16, tag="tB")
            nc.vector.tensor_tensor(Waw[:], W[:], aw_h[:, HALO_L:HALO_L + N],
                                    Alu.mult)

            # ---- focal factors -----------------------------------------
            AOO = val.tile([P, N], dt.bfloat16, tag="tC")
            nc.vector._custom_dve(ada_op, out=AOO[:], in0=sg[:], in1=pt[:],
                                  s0=0.5)
            lhs = val.tile([P, N], dt.bfloat16, tag="tD")
            nc.vector.tensor_tensor(lhs[:], AOO[:], Waw[:], Alu.mult)
            rhs = bce
            if with_mask:
                mbv = mt[:].bitcast(dt.bfloat16)[:, 1::2]
                bm = val.tile([P, N], dt.bfloat16, tag="bm")
                nc.vector.tensor_tensor(bm[:], bce[:], mbv, Alu.mult)
                rhs = bm
                nc.vector.tensor_reduce(
                    ms[:, i:i + 1], mt[:], axis=mybir.AxisListType.X, op=Alu.add)

            # ---- PE contraction: acc[m,n] += sum_b lhs[b,m]*rhs[b,n] ---
            for c in range(n_chunks):
                s0 = c * CH
                nc.tensor.matmul(
                    acc[0:CH, 0:CH],
                    lhs[:, s0:s0 + CH],
                    rhs[:, s0:s0 + CH],
                    start=(i == 0 and c == 0),
                    stop=(i == NT - 1 and c == n_chunks - 1),
                )

        # ---- tail: diagonal of acc holds per-column sums ---------------
        accsb = singles.tile([P, CH], dt.float32)
        nc.vector.tensor_copy(accsb[0:CH, :], acc[0:CH, 0:CH])
        diag = singles.tile([P, CH], dt.float32)
        nc.vector.tensor_tensor(
            diag[0:CH, :], accsb[0:CH, :], eye_sb[0:CH, 0:CH], Alu.mult)
        nc.vector.tensor_reduce(out_sb[0:CH, 0:1], diag[0:CH, :],
                                axis=mybir.AxisListType.X, op=Alu.add)
        if with_mask:
            nc.vector.tensor_reduce(
                out_sb[:, 1:2], ms[:], axis=mybir.AxisListType.X, op=Alu.add)
        nc.sync.dma_start(out_d[:], out_sb[:])

    nc.compile()
    return nc


_PROGRAM_CACHE = {}


def _get_program(T_shard, N=2500, with_mask=False):
    key = (T_shard, N, with_mask)
    if key not in _PROGRAM_CACHE:
        _PROGRAM_CACHE[key] = _build_program(T_shard, N, with_mask)
    return _PROGRAM_CACHE[key]


def _make_in_maps(x, t, m=None):
    """Per-core input dicts (plain f32/i32 slices; views happen on-device)."""
    Bq, T = x.shape
    T_shard = T // N_CORES
    t_pad = np.pad(t, ((0, 0), (HALO_L, HALO_R)), mode="edge")
    eye = np.eye(P, dtype=np.float32)
    in_maps = []
    for c in range(N_CORES):
        lo = c * T_shard
        im = {
            "x": np.ascontiguousarray(x[:, lo:lo + T_shard]),
            "t": np.ascontiguousarray(t_pad[:, lo:lo + T_shard + HALO]),
            "eye": eye,
        }
        if m is not None:
            im["m"] = np.ascontiguousarray(m[:, lo:lo + T_shard])
        in_maps.append(im)
    return in_maps


def kernel(inputs, targets, mask):
    from concourse.bass_utils import run_bass_kernel_spmd

    x = np.ascontiguousarray(np.asarray(inputs, dtype=np.float32))
    t = np.ascontiguousarray(np.asarray(targets, dtype=np.int32))
    m = np.ascontiguousarray(np.asarray(mask, dtype=np.float32))
    Bq, T = x.shape
    assert Bq == P and T % N_CORES == 0
    T_shard = T // N_CORES
    ones_mask = bool(m.min() == 1.0 and m.max() == 1.0)

    nc = _get_program(T_shard, 2500, with_mask=not ones_mask)
    in_maps = _make_in_maps(x, t, None if ones_mask else m)

    res = run_bass_kernel_spmd(nc, in_maps, core_ids=list(range(N_CORES)))
    outs = [r["out"] for r in res.results]

    loss = float(sum(o[:, 0].astype(np.float64).sum() for o in outs))
    if ones_mask:
        msum = float(Bq) * float(T)
    else:
        msum = float(sum(o[:, 1].astype(np.float64).sum() for o in outs))
    if msum <= 0.0:
        return np.float32(0.0)
    return np.float32(loss / msum)
